# revision 1
# baseline (speedup 1.0000x reference)
"""Trainium2 Bass kernel for masked BasicBlock (conv3x3+BN+ReLU, gated, x2, residual).

Data-parallel over batch: 8 images -> 8 NeuronCores. Per core, NCHW [64,256,256]
in 8 row-strips of 32 output rows:
  - conv3x3 = 9 accumulated matmuls over C_in=64. Taps (dy=-1,dx)/(dy=+1,dx)
    are K-packed to 128 partitions via a 2-row-shifted duplicate of the input
    in partitions 64..127 (3 K=128 matmuls + 3 K=64 per chunk); chunk pairs
    (A|B = 4 consecutive rows) run concurrently on the two PE column groups
    via tile_position (0,0)/(0,64).
  - The 2-row shift also makes T1[0:128] directly usable as the residual pair.
  - Gating masks are broadcast to all partitions with K=1 ones-matmuls from a
    flat bf16 mask (PE->PSUM), not DMA chains.
  - BN(+ReLU) on ScalarE from PSUM; final relu on ScalarE; elementwise gating
    and residual on VectorE; strip-level staging tiles keep DMA count ~10/strip.
"""
import sys
import os

sys.path.insert(0, '/opt/trn_rl_repo')

import numpy as np
import ml_dtypes

BF16 = ml_dtypes.bfloat16

B, C, H, W = 8, 64, 256, 256
WP = W + 2           # padded row width
R = 32               # output rows per strip
NS = H // R          # strips
NP1 = (R + 4) // 4   # conv1 pairs per strip (h rows r0-1 .. r0+34)
NP2 = R // 4         # conv2 pairs per strip
XR = R + 6           # x rows per strip: [r0-2, r0+36)
HR = R + 4           # h rows per strip: [r0-1, r0+35)
PAD = 4              # zero rows padded above/below the flat masks
GMR = HR + 1         # gmax flat rows loaded per strip
GTR = R + 1          # gate flat rows loaded per strip

_CACHE = {}


def _build(iters=1):
    import concourse.bacc as bacc_mod
    import concourse.tile as tile
    import concourse.mybir as mybir

    dt = mybir.dt
    nc = bacc_mod.Bacc()

    x_d = nc.dram_tensor("x", [C, H, W], dt.float32, kind="ExternalInput")
    gmx_d = nc.dram_tensor("gmx", [(H + 2 * PAD) * W], dt.bfloat16, kind="ExternalInput")
    gt_d = nc.dram_tensor("gt", [(H + 2 * PAD) * W], dt.bfloat16, kind="ExternalInput")
    wp1_d = nc.dram_tensor("wp1", [128, 3, 64], dt.bfloat16, kind="ExternalInput")
    ws1_d = nc.dram_tensor("ws1", [64, 3, 64], dt.bfloat16, kind="ExternalInput")
    wp2_d = nc.dram_tensor("wp2", [128, 3, 64], dt.bfloat16, kind="ExternalInput")
    ws2_d = nc.dram_tensor("ws2", [64, 3, 64], dt.bfloat16, kind="ExternalInput")
    sb1_d = nc.dram_tensor("sb1", [128, 2], dt.float32, kind="ExternalInput")
    sb2_d = nc.dram_tensor("sb2", [128, 2], dt.float32, kind="ExternalInput")
    o_d = nc.dram_tensor("o", [C, H, W], dt.float32, kind="ExternalOutput")

    with tile.TileContext(nc) as tc:
        with (
            tc.tile_pool(name="const", bufs=1) as cpool,
            tc.tile_pool(name="xs", bufs=3) as xpool,
            tc.tile_pool(name="hs", bufs=1) as hpool,
            tc.tile_pool(name="stage", bufs=2) as spool,
            tc.tile_pool(name="ov", bufs=1) as ovpool,
            tc.tile_pool(name="flat", bufs=2) as fpool,
            tc.tile_pool(name="pair", bufs=3) as ppool,
            tc.tile_pool(name="ps1", bufs=2, space="PSUM") as ps1,
            tc.tile_pool(name="ps2", bufs=2, space="PSUM") as ps2,
            tc.tile_pool(name="pmA", bufs=2, space="PSUM") as pmA,
            tc.tile_pool(name="pmB", bufs=2, space="PSUM") as pmB,
        ):
            wp1 = cpool.tile([128, 3, 64], dt.bfloat16)
            ws1 = cpool.tile([64, 3, 64], dt.bfloat16)
            wp2 = cpool.tile([128, 3, 64], dt.bfloat16)
            ws2 = cpool.tile([64, 3, 64], dt.bfloat16)
            sb1 = cpool.tile([128, 2], dt.float32)
            sb2 = cpool.tile([128, 2], dt.float32)
            ones = cpool.tile([1, 128], dt.bfloat16)
            nc.sync.dma_start(wp1[:], wp1_d[:])
            nc.sync.dma_start(ws1[:], ws1_d[:])
            nc.sync.dma_start(wp2[:], wp2_d[:])
            nc.sync.dma_start(ws2[:], ws2_d[:])
            nc.sync.dma_start(sb1[:], sb1_d[:])
            nc.sync.dma_start(sb2[:], sb2_d[:])
            nc.vector.memset(ones[:], 1.0)

            for it_s in range(iters * NS):
                s = it_s % NS
                r0 = s * R
                # ---- x strip: T1 [128, XR, WP] bf16; lower=x padded; upper=x shifted +2 rows
                T1 = xpool.tile([128, XR, WP], dt.bfloat16, tag="T1")
                first = r0 - 2
                v0 = max(0, -first)
                v1 = min(XR, H - first)
                nc.vector.memset(T1[0:64, :, 0:1], 0)
                nc.vector.memset(T1[0:64, :, 257:258], 0)
                if v0 > 0:
                    nc.vector.memset(T1[0:64, 0:v0, :], 0)
                if v1 < XR:
                    nc.vector.memset(T1[0:64, v1:XR, :], 0)
                nc.gpsimd.dma_start(T1[0:64, v0:v1, 1:257], x_d[:, first + v0:first + v1, :])
                # upper[i] = lower[i+2]  (x shifted by +2 rows, same cols)
                nc.scalar.dma_start(T1[64:128, 0:XR - 2, :], T1[0:64, 2:XR, :])

                # ---- flat mask strips (single partition, bf16)
                gmaxf = fpool.tile([1, GMR * W], dt.bfloat16, tag="gmaxf")
                gatef = fpool.tile([1, GTR * W], dt.bfloat16, tag="gatef")
                gm0 = (r0 + 3) * W
                gt0 = (r0 + PAD) * W
                nc.sync.dma_start(gmaxf[0:1, :], gmx_d[gm0:gm0 + GMR * W].unsqueeze(0))
                nc.sync.dma_start(gatef[0:1, :], gt_d[gt0:gt0 + GTR * W].unsqueeze(0))

                # ---- h strip + staging
                H1 = hpool.tile([128, HR, WP], dt.bfloat16, tag="H1")
                HP = spool.tile([128, NP1, 512], dt.bfloat16, tag="HP")
                OV = ovpool.tile([128, NP2, 512], dt.float32, tag="OV")
                nc.vector.memset(H1[:, :, 0:1], 0)
                nc.vector.memset(H1[:, :, 257:258], 0)

                # ---- conv1: 9 pairs of 4 h-rows
                for pq in range(NP1):
                    gm_ps = pmA.tile([128, 512], dt.float32, tag="pmA")
                    offA = 4 * pq * W
                    nc.tensor.matmul(gm_ps[0:64, :], ones[0:1, 0:64], gmaxf[0:1, offA:offA + 512],
                                     start=True, stop=True, tile_position=(0, 0), skip_group_check=True)
                    nc.tensor.matmul(gm_ps[64:128, :], ones[0:1, 64:128], gmaxf[0:1, offA + 512:offA + 1024],
                                     start=True, stop=True, tile_position=(0, 64), skip_group_check=True)

                    acc = ps1.tile([128, 512], dt.float32, tag="ps1")
                    i0 = 4 * pq + 1
                    for dx in range(3):
                        nc.tensor.matmul(acc[0:64, :], wp1[:, dx, :], T1[:, i0 - 1:i0 + 1, dx:dx + 256],
                                         start=(dx == 0), stop=False, tile_position=(0, 0), skip_group_check=True)
                        nc.tensor.matmul(acc[64:128, :], wp1[:, dx, :], T1[:, i0 + 1:i0 + 3, dx:dx + 256],
                                         start=(dx == 0), stop=False, tile_position=(0, 64), skip_group_check=True)
                    for dx in range(3):
                        nc.tensor.matmul(acc[0:64, :], ws1[:, dx, :], T1[0:64, i0:i0 + 2, dx:dx + 256],
                                         start=False, stop=(dx == 2), tile_position=(0, 0), skip_group_check=True)
                        nc.tensor.matmul(acc[64:128, :], ws1[:, dx, :], T1[0:64, i0 + 2:i0 + 4, dx:dx + 256],
                                         start=False, stop=(dx == 2), tile_position=(0, 64), skip_group_check=True)
                    st = ppool.tile([128, 512], dt.bfloat16, tag="st")
                    nc.scalar.activation(st[:], acc[:], mybir.ActivationFunctionType.Relu,
                                         bias=sb1[:, 1:2], scale=sb1[:, 0:1])
                    nc.vector.tensor_tensor(HP[:, pq, :], st[:], gm_ps[:], mybir.AluOpType.mult)

                # distribute HP -> H1 (lower = h, upper = h shifted +2 rows)
                # (DMA APs max 3 dims: one DMA per row-in-pair)
                h1v = H1[0:64, 0:4 * NP1, 1:257].rearrange("c (p rr) w -> c p rr w", p=NP1)
                h1u = H1[64:128, 2:2 + 4 * (NP1 - 1), 1:257].rearrange("c (p rr) w -> c p rr w", p=NP1 - 1)
                h1u2 = H1[64:128, 0:4 * NP1, 1:257].rearrange("c (p rr) w -> c p rr w", p=NP1)
                hpv = HP[:].rearrange("c p (rr w) -> c p rr w", rr=2)
                for rr in range(2):
                    nc.sync.dma_start(h1v[:, :, rr, :], hpv[0:64, :, rr, :])
                    nc.sync.dma_start(h1v[:, :, 2 + rr, :], hpv[64:128, :, rr, :])
                    nc.sync.dma_start(h1u[:, :, rr, :], hpv[0:64, 1:NP1, rr, :])
                    nc.sync.dma_start(h1u2[:, :, rr, :], hpv[64:128, :, rr, :])

                # ---- conv2: 8 pairs of 4 output rows
                for q in range(NP2):
                    gt_ps = pmB.tile([128, 512], dt.float32, tag="pmB")
                    offA = 4 * q * W
                    nc.tensor.matmul(gt_ps[0:64, :], ones[0:1, 0:64], gatef[0:1, offA:offA + 512],
                                     start=True, stop=True, tile_position=(0, 0), skip_group_check=True)
                    nc.tensor.matmul(gt_ps[64:128, :], ones[0:1, 64:128], gatef[0:1, offA + 512:offA + 1024],
                                     start=True, stop=True, tile_position=(0, 64), skip_group_check=True)

                    acc2 = ps2.tile([128, 512], dt.float32, tag="ps2")
                    m0 = 4 * q + 1
                    for dx in range(3):
                        nc.tensor.matmul(acc2[0:64, :], wp2[:, dx, :], H1[:, m0 - 1:m0 + 1, dx:dx + 256],
                                         start=(dx == 0), stop=False, tile_position=(0, 0), skip_group_check=True)
                        nc.tensor.matmul(acc2[64:128, :], wp2[:, dx, :], H1[:, m0 + 1:m0 + 3, dx:dx + 256],
                                         start=(dx == 0), stop=False, tile_position=(0, 64), skip_group_check=True)
                    for dx in range(3):
                        nc.tensor.matmul(acc2[0:64, :], ws2[:, dx, :], H1[0:64, m0:m0 + 2, dx:dx + 256],
                                         start=False, stop=(dx == 2), tile_position=(0, 0), skip_group_check=True)
                        nc.tensor.matmul(acc2[64:128, :], ws2[:, dx, :], H1[0:64, m0 + 2:m0 + 4, dx:dx + 256],
                                         start=False, stop=(dx == 2), tile_position=(0, 64), skip_group_check=True)
                    u2 = ppool.tile([128, 512], dt.float32, tag="u2")
                    nc.scalar.activation(u2[:], acc2[:], mybir.ActivationFunctionType.Identity,
                                         bias=sb2[:, 1:2], scale=sb2[:, 0:1])
                    t = ppool.tile([128, 512], dt.float32, tag="t")
                    nc.vector.tensor_tensor(t[:], u2[:], gt_ps[:], mybir.AluOpType.mult)
                    lz = 4 * q + 2
                    v = ppool.tile([128, 512], dt.float32, tag="v")
                    nc.vector.tensor_tensor(v[:].rearrange("p (r w) -> p r w", r=2),
                                            t[:].rearrange("p (r w) -> p r w", r=2),
                                            T1[:, lz:lz + 2, 1:257], mybir.AluOpType.add)
                    nc.scalar.activation(OV[:, q, :], v[:], mybir.ActivationFunctionType.Relu)

                ov = o_d[:, r0:r0 + R, :].rearrange("c (p rr) w -> c p rr w", p=NP2)
                ovv = OV[:].rearrange("c p (rr w) -> c p rr w", rr=2)
                for rr in range(2):
                    nc.scalar.dma_start(ov[:, :, rr, :], ovv[0:64, :, rr, :])
                    nc.scalar.dma_start(ov[:, :, 2 + rr, :], ovv[64:128, :, rr, :])
    nc.finalize()
    return nc


def _host_prep(gate, w1, scale1, bias1, w2, scale2, bias2):
    # weights: lhsT[ci, co] = w[co, ci, dy, dx]; K-pack dy=-1 (lower) with dy=+1 (upper)
    def pack(w):
        wt = np.transpose(w, (1, 0, 2, 3))  # [ci, co, dy, dx]
        wp = np.empty((128, 3, 64), np.float32)
        ws = np.empty((64, 3, 64), np.float32)
        for dx in range(3):
            wp[0:64, dx] = wt[:, :, 0, dx]
            wp[64:128, dx] = wt[:, :, 2, dx]
            ws[:, dx] = wt[:, :, 1, dx]
        return wp.astype(BF16), ws.astype(BF16)

    wp1, ws1 = pack(w1)
    wp2, ws2 = pack(w2)
    sb1 = np.stack([np.tile(scale1, 2), np.tile(bias1, 2)], axis=1).astype(np.float32)
    sb2 = np.stack([np.tile(scale2, 2), np.tile(bias2, 2)], axis=1).astype(np.float32)

    def flat_padded(m):
        mp = np.zeros((H + 2 * PAD, W), np.float32)
        mp[PAD:PAD + H] = m
        return mp.reshape(-1).astype(BF16)

    gmx_list, gt_list = [], []
    for b in range(B):
        g = gate[b, 0]
        gp = np.pad(g, 1)
        gm = np.zeros_like(g)
        for dy in range(3):
            for dx in range(3):
                np.maximum(gm, gp[dy:dy + H, dx:dx + W], out=gm)
        gmx_list.append(flat_padded(gm))
        gt_list.append(flat_padded(g))
    return wp1, ws1, wp2, ws2, sb1, sb2, gmx_list, gt_list


def kernel(x, gate, w1, scale1, bias1, w2, scale2, bias2):
    from concourse.bass_utils import run_bass_kernel_spmd

    x = np.asarray(x, np.float32)
    gate = np.asarray(gate, np.float32)
    wp1, ws1, wp2, ws2, sb1, sb2, gmx_list, gt_list = _host_prep(
        gate, np.asarray(w1, np.float32), np.asarray(scale1, np.float32),
        np.asarray(bias1, np.float32), np.asarray(w2, np.float32),
        np.asarray(scale2, np.float32), np.asarray(bias2, np.float32))

    if 'nc' not in _CACHE:
        _CACHE['nc'] = _build()
    nc = _CACHE['nc']

    in_maps = []
    for b in range(B):
        in_maps.append({
            "x": np.ascontiguousarray(x[b]),
            "gmx": gmx_list[b], "gt": gt_list[b],
            "wp1": wp1, "ws1": ws1, "wp2": wp2, "ws2": ws2,
            "sb1": sb1, "sb2": sb2,
        })
    res = run_bass_kernel_spmd(nc, in_maps, core_ids=list(range(B)))
    _CACHE['last_results'] = res
    out = np.stack([res.results[b]["o"] for b in range(B)], axis=0)
    return out



# revision 3
# speedup vs baseline: 1.7853x; 1.7853x over previous
"""Trainium2 Bass kernel v4 for masked BasicBlock (conv3x3+BN+ReLU, gated, x2, residual).

Data-parallel over batch: 8 images -> 8 NeuronCores. Per core, NCHW [64,256,256]
in 8 row-strips of 32 output rows.

Core idea (v4): every conv tap (dy,dx) is ONE [128,512] matmul with a
block-diagonal lhsT — lower 64 partitions of the rhs hold rows for output
group A, upper 64 hold rows (shifted +2) for group B, and the 64x64 tap weight
matrix sits on both diagonal blocks. 9 taps + 1 gating-selector matmul per
4-row pair, for both convs. The selector (K=2, fp8) accumulates BIG*gmax into
conv1's PSUM (ReLU with bias-BIG*scale clamps inactive pixels to 0) and
broadcasts the gate for conv2's multiply.

conv1's ACT writes h straight into the H1 conv layout (two 64-partition
activations); two strip-level SBUF DMAs patch the cross-partition quarters.
Strips are software-pipelined; x in/out are bf16 on the wire.
"""
import sys

sys.path.insert(0, '/opt/trn_rl_repo')

import numpy as np
import ml_dtypes

BF16 = ml_dtypes.bfloat16
FP8 = ml_dtypes.float8_e4m3fn

B, C, H, W = 8, 64, 256, 256
WP = W + 2           # padded row width
R = 32               # output rows per strip
NS = H // R          # strips
NP1 = (R + 4) // 4   # conv1 pairs per strip (h rows r0-1 .. r0+34)
NP2 = R // 4         # conv2 pairs per strip
XR = R + 6           # x rows per strip: [r0-2, r0+36)
HR = R + 4           # h rows per strip: [r0-1, r0+35)
GPAD = 5             # gmax pad rows on top (1 mod 4 so pair groups are 4-row aligned)
GROWS = GPAD + H + 3 # 264 gmax padded rows
NSB = GROWS // 4     # 66 gmax super-blocks (4 rows = 2x512 blocks)
NSB2 = H // 4        # 64 gate super-blocks
BIG = 64.0

_CACHE = {}


def _build(iters=1):
    import concourse.bacc as bacc_mod
    import concourse.tile as tile
    import concourse.mybir as mybir

    dt = mybir.dt
    nc = bacc_mod.Bacc()

    x_d = nc.dram_tensor("x", [C, H, W], dt.bfloat16, kind="ExternalInput")
    gm2_d = nc.dram_tensor("gm2", [2, NSB * 512], dt.float8e4, kind="ExternalInput")
    gt2_d = nc.dram_tensor("gt2", [2, NSB2 * 512], dt.float8e4, kind="ExternalInput")
    wd1_d = nc.dram_tensor("wd1", [128, 9, 128], dt.bfloat16, kind="ExternalInput")
    wd2_d = nc.dram_tensor("wd2", [128, 9, 128], dt.bfloat16, kind="ExternalInput")
    sb1_d = nc.dram_tensor("sb1", [128, 2], dt.float32, kind="ExternalInput")
    sb2_d = nc.dram_tensor("sb2", [128, 2], dt.float32, kind="ExternalInput")
    selb_d = nc.dram_tensor("selb", [2, 128], dt.float8e4, kind="ExternalInput")
    selp_d = nc.dram_tensor("selp", [2, 128], dt.float8e4, kind="ExternalInput")
    o_d = nc.dram_tensor("o", [C, H, W], dt.bfloat16, kind="ExternalOutput")

    RELU = mybir.ActivationFunctionType.Relu
    IDENT = mybir.ActivationFunctionType.Identity

    with tile.TileContext(nc) as tc:
        with (
            tc.tile_pool(name="const", bufs=1) as cpool,
            tc.tile_pool(name="xs", bufs=3) as xpool,
            tc.tile_pool(name="hs", bufs=3) as hpool,
            tc.tile_pool(name="msk", bufs=2) as mpool,
            tc.tile_pool(name="ov", bufs=2) as ovpool,
            tc.tile_pool(name="work", bufs=2) as wpool,
            tc.tile_pool(name="ps1", bufs=2, space="PSUM") as ps1,
            tc.tile_pool(name="ps2", bufs=3, space="PSUM") as ps2,
            tc.tile_pool(name="pm", bufs=3, space="PSUM") as pmp,
        ):
            wd1 = cpool.tile([128, 9, 128], dt.bfloat16)
            wd2 = cpool.tile([128, 9, 128], dt.bfloat16)
            sb1 = cpool.tile([128, 2], dt.float32)
            sb2 = cpool.tile([128, 2], dt.float32)
            selb = cpool.tile([2, 128], dt.float8e4)
            selp = cpool.tile([2, 128], dt.float8e4)
            for t, d in ((wd1, wd1_d), (wd2, wd2_d), (sb1, sb1_d), (sb2, sb2_d),
                         (selb, selb_d), (selp, selp_d)):
                nc.sync.dma_start(t[:], d[:])
            warm = cpool.tile([2, 64], dt.bfloat16)
            nc.vector.memset(warm[:], 0)
            wps = ps2.tile([128, 512], dt.float32, tag="ps2")
            for _ in range(60):
                nc.tensor.matmul(wps[0:64, 0:64], warm[:, 0:64], warm[:, :],
                                 start=True, stop=True, tile_position=(0, 0), skip_group_check=True)

            def emit_load(s):
                r0 = s * R
                first = r0 - 2
                T1 = xpool.tile([128, XR, WP], dt.bfloat16, tag="T1")
                v0 = max(0, -first)
                v1 = min(XR, H - first)
                v1u = min(XR, H - first - 2)
                nc.vector.memset(T1[:, :, 0:1], 0)
                nc.vector.memset(T1[:, :, 257:258], 0)
                if v0 > 0:
                    nc.vector.memset(T1[0:64, 0:v0, :], 0)
                if v1 < XR:
                    nc.vector.memset(T1[0:64, v1:XR, :], 0)
                if v1u < XR:
                    nc.vector.memset(T1[64:128, v1u:XR, :], 0)
                if s == 0:
                    vh = 14
                    nc.gpsimd.dma_start(T1[0:64, v0:vh, 1:257], x_d[:, first + v0:first + vh, :])
                    nc.scalar.dma_start(T1[64:128, 0:vh, 1:257], x_d[:, first + 2:first + 2 + vh, :])
                    nc.gpsimd.dma_start(T1[0:64, vh:v1, 1:257], x_d[:, first + vh:first + v1, :])
                    nc.scalar.dma_start(T1[64:128, vh:v1u, 1:257], x_d[:, first + 2 + vh:first + 2 + v1u, :])
                else:
                    nc.gpsimd.dma_start(T1[0:64, v0:v1, 1:257], x_d[:, first + v0:first + v1, :])
                    nc.scalar.dma_start(T1[64:128, 0:v1u, 1:257], x_d[:, first + 2:first + 2 + v1u, :])
                gmw = mpool.tile([2, NP1 * 512], dt.float8e4, tag="gmw")
                gtw = mpool.tile([2, NP2 * 512], dt.float8e4, tag="gtw")
                S0 = r0 // 4 + 1
                nc.sync.dma_start(gmw[:], gm2_d[0:2, S0 * 512:(S0 + NP1) * 512])
                nc.sync.dma_start(gtw[:], gt2_d[0:2, (r0 // 4) * 512:(r0 // 4 + NP2) * 512])
                return T1, gmw, gtw

            def new_H1():
                H1 = hpool.tile([128, HR, WP], dt.bfloat16, tag="H1")
                nc.vector.memset(H1[:, :, 0:1], 0)
                nc.vector.memset(H1[:, :, 257:258], 0)
                return H1

            def emit_conv1(s, T1, gmw, H1s):
                # computes global pairs 8s+pq for pq in [0..8] (s==0) or [1..8];
                # the boundary pair (pq==8) also writes rows 0:4 of H1(s+1).
                H1 = H1s[s]
                for pq in (range(NP1) if s == 0 else range(1, NP1)):
                    acc = ps1.tile([128, 512], dt.float32, tag="ps1")
                    for k in range(9):
                        dy, dx = k // 3, k % 3
                        tt = 4 * pq + dy
                        nc.tensor.matmul(acc[:, :], wd1[:, k, :], T1[:, tt:tt + 2, dx:dx + 256],
                                         start=(k == 0), stop=False, tile_position=(0, 0),
                                         skip_group_check=True)
                    nc.tensor.matmul(acc[:, :], selb[:, :], gmw[0:2, pq * 512:pq * 512 + 512],
                                     start=False, stop=True, tile_position=(0, 0), skip_group_check=True)
                    accv = acc[:].rearrange("p (r w) -> p r w", r=2)
                    nc.scalar.activation(H1[0:64, 4 * pq:4 * pq + 2, 1:257], accv[0:64],
                                         RELU, bias=sb1[0:64, 1:2], scale=sb1[0:64, 0:1])
                    nc.scalar.activation(H1[64:128, 4 * pq:4 * pq + 2, 1:257], accv[64:128],
                                         RELU, bias=sb1[64:128, 1:2], scale=sb1[64:128, 0:1])
                    if pq == NP1 - 1 and s + 1 < NS:
                        H1n = new_H1()
                        H1s[s + 1] = H1n
                        nc.scalar.activation(H1n[0:64, 0:2, 1:257], accv[0:64],
                                             RELU, bias=sb1[0:64, 1:2], scale=sb1[0:64, 0:1])
                        nc.scalar.activation(H1n[64:128, 0:2, 1:257], accv[64:128],
                                             RELU, bias=sb1[64:128, 1:2], scale=sb1[64:128, 0:1])
                # patch the two cross-partition quarters:
                h1l4 = H1[0:64].rearrange("c (p a) w -> c p (a w)", a=4)
                h1u4 = H1[64:128].rearrange("c (p a) w -> c p (a w)", a=4)
                nc.sync.dma_start(h1l4[:, :, 2 * WP:4 * WP], h1u4[:, :, 0:2 * WP])
                nc.sync.dma_start(h1u4[:, 0:NP1 - 1, 2 * WP:4 * WP], h1l4[:, 1:NP1, 0:2 * WP])

            def emit_conv2(s, T1, H1, gtw):
                r0 = s * R
                OV = ovpool.tile([128, NP2, 512], dt.bfloat16, tag="OV")
                for q in range(NP2):
                    pm = pmp.tile([128, 512], dt.float32, tag="pm")
                    nc.tensor.matmul(pm[:, :], selp[:, :], gtw[0:2, q * 512:q * 512 + 512],
                                     start=True, stop=True, tile_position=(0, 0), skip_group_check=True)
                    acc2 = ps2.tile([128, 512], dt.float32, tag="ps2")
                    for k in range(9):
                        dy, dx = k // 3, k % 3
                        mm = 4 * q + dy
                        nc.tensor.matmul(acc2[:, :], wd2[:, k, :], H1[:, mm:mm + 2, dx:dx + 256],
                                         start=(k == 0), stop=(k == 8), tile_position=(0, 0),
                                         skip_group_check=True)
                    u2 = wpool.tile([128, 512], dt.bfloat16, tag="u2")
                    nc.vector.tensor_scalar(u2[:], acc2[:], sb2[:, 0:1], sb2[:, 1:2],
                                            mybir.AluOpType.mult, mybir.AluOpType.add)
                    t = wpool.tile([128, 512], dt.bfloat16, tag="t")
                    nc.vector.tensor_tensor(t[:], u2[:], pm[:], mybir.AluOpType.mult)
                    v = wpool.tile([128, 512], dt.bfloat16, tag="v")
                    lz = 4 * q + 2
                    nc.vector.tensor_tensor(v[:].rearrange("p (r w) -> p r w", r=2),
                                            t[:].rearrange("p (r w) -> p r w", r=2),
                                            T1[:, lz:lz + 2, 1:257], mybir.AluOpType.add)
                    nc.scalar.activation(OV[:, q, :], v[:], RELU)
                o4 = o_d[:, r0:r0 + R, :].rearrange("c (q a) w -> c q (a w)", a=4)
                nc.gpsimd.dma_start(o4[:, 0:4, 0:512], OV[0:64, 0:4, :])
                nc.sync.dma_start(o4[:, 0:4, 512:1024], OV[64:128, 0:4, :])
                nc.gpsimd.dma_start(o4[:, 4:8, 0:512], OV[0:64, 4:8, :])
                nc.sync.dma_start(o4[:, 4:8, 512:1024], OV[64:128, 4:8, :])

            for it in range(iters):
                T1s = {}
                H1s = {}
                T1s[0] = emit_load(0)
                H1s[0] = new_H1()
                emit_conv1(0, T1s[0][0], T1s[0][1], H1s)
                for s in range(NS):
                    if s + 1 < NS:
                        T1s[s + 1] = emit_load(s + 1)
                        emit_conv1(s + 1, T1s[s + 1][0], T1s[s + 1][1], H1s)
                    emit_conv2(s, T1s[s][0], H1s[s], T1s[s][2])
                    T1s.pop(s)
                    H1s.pop(s)
    nc.finalize()
    return nc


def _host_prep(x, gate, w1, scale1, bias1, w2, scale2, bias2):
    # wd[k]: block-diagonal [128,128], diag blocks = wt[:, :, dy, dx] (tap k = 3*dy+dx)
    def pack(w):
        wt = np.transpose(w, (1, 0, 2, 3))  # [ci, co, dy, dx]
        wd = np.zeros((128, 9, 128), np.float32)
        for k in range(9):
            dy, dx = k // 3, k % 3
            wd[0:64, k, 0:64] = wt[:, :, dy, dx]
            wd[64:128, k, 64:128] = wt[:, :, dy, dx]
        return wd.astype(BF16)

    wd1 = pack(w1)
    wd2 = pack(w2)
    # conv1 bias folded with the -BIG gate clamp: relu(s*(acc + BIG*g) + b - s*BIG)
    sb1 = np.stack([np.tile(scale1, 2), np.tile(bias1 - scale1 * BIG, 2)], axis=1).astype(np.float32)
    sb2 = np.stack([np.tile(scale2, 2), np.tile(bias2, 2)], axis=1).astype(np.float32)

    selb = np.zeros((2, 128), np.float32)
    selb[0, 0:64] = BIG
    selb[1, 64:128] = BIG
    selp = np.zeros((2, 128), np.float32)
    selp[0, 0:64] = 1.0
    selp[1, 64:128] = 1.0
    selb = selb.astype(FP8)
    selp = selp.astype(FP8)

    g = gate[:, 0]                                   # [B, H, W]
    gp = np.pad(g, ((0, 0), (1, 1), (1, 1)))
    gm = np.zeros_like(g)
    for dy in range(3):
        for dx in range(3):
            np.maximum(gm, gp[:, dy:dy + H, dx:dx + W], out=gm)

    def blocks2(padded):                             # [rows(4k), W] -> [2, k*512]
        nsb = padded.shape[0] // 4
        arr = padded.reshape(nsb, 2, 512)
        return np.ascontiguousarray(arr.transpose(1, 0, 2)).reshape(2, -1).astype(FP8)

    gm2_l, gt2_l, xbf = [], [], []
    for bi in range(B):
        gmp = np.zeros((GROWS, W), np.float32)
        gmp[GPAD:GPAD + H] = gm[bi]
        gm2_l.append(blocks2(gmp))
        gt2_l.append(blocks2(g[bi]))
        xbf.append(np.ascontiguousarray(x[bi]).astype(BF16))
    return dict(wd1=wd1, wd2=wd2, sb1=sb1, sb2=sb2,
                selb=selb, selp=selp, gm2=gm2_l, gt2=gt2_l, x=xbf)


def _in_map(prep, bi):
    return {
        "x": prep["x"][bi],
        "gm2": prep["gm2"][bi], "gt2": prep["gt2"][bi],
        "wd1": prep["wd1"], "wd2": prep["wd2"],
        "sb1": prep["sb1"], "sb2": prep["sb2"], "selb": prep["selb"], "selp": prep["selp"],
    }


def kernel(x, gate, w1, scale1, bias1, w2, scale2, bias2):
    from concourse.bass_utils import run_bass_kernel_spmd

    x = np.asarray(x, np.float32)
    gate = np.asarray(gate, np.float32)
    prep = _host_prep(x, gate, np.asarray(w1, np.float32), np.asarray(scale1, np.float32),
                      np.asarray(bias1, np.float32), np.asarray(w2, np.float32),
                      np.asarray(scale2, np.float32), np.asarray(bias2, np.float32))

    if 'nc' not in _CACHE:
        _CACHE['nc'] = _build()
    nc = _CACHE['nc']

    in_maps = [_in_map(prep, bi) for bi in range(B)]
    res = run_bass_kernel_spmd(nc, in_maps, core_ids=list(range(B)))
    _CACHE['last_results'] = res
    out = np.stack([res.results[bi]["o"].astype(np.float32) for bi in range(B)], axis=0)
    return out


# revision 4
# speedup vs baseline: 1.8301x; 1.0251x over previous
"""Trainium2 Bass kernel v4 for masked BasicBlock (conv3x3+BN+ReLU, gated, x2, residual).

Data-parallel over batch: 8 images -> 8 NeuronCores. Per core, NCHW [64,256,256]
in 8 row-strips of 32 output rows.

Core idea (v4): every conv tap (dy,dx) is ONE [128,512] matmul with a
block-diagonal lhsT — lower 64 partitions of the rhs hold rows for output
group A, upper 64 hold rows (shifted +2) for group B, and the 64x64 tap weight
matrix sits on both diagonal blocks. 9 taps + 1 gating-selector matmul per
4-row pair, for both convs. The selector (K=2, fp8) accumulates BIG*gmax into
conv1's PSUM (ReLU with bias-BIG*scale clamps inactive pixels to 0) and
broadcasts the gate for conv2's multiply.

conv1's ACT writes h straight into the H1 conv layout (two 64-partition
activations); two strip-level SBUF DMAs patch the cross-partition quarters.
Strips are software-pipelined; x in/out are bf16 on the wire.
"""
import sys

sys.path.insert(0, '/opt/trn_rl_repo')

import numpy as np
import ml_dtypes

BF16 = ml_dtypes.bfloat16
FP8 = ml_dtypes.float8_e4m3fn

B, C, H, W = 8, 64, 256, 256
WP = W + 2           # padded row width
R = 32               # output rows per strip
NS = H // R          # strips
NP1 = (R + 4) // 4   # conv1 pairs per strip (h rows r0-1 .. r0+34)
NP2 = R // 4         # conv2 pairs per strip
XR = R + 6           # x rows per strip: [r0-2, r0+36)
HR = R + 4           # h rows per strip: [r0-1, r0+35)
GPAD = 5             # gmax pad rows on top (1 mod 4 so pair groups are 4-row aligned)
GROWS = GPAD + H + 3 # 264 gmax padded rows
NSB = GROWS // 4     # 66 gmax super-blocks (4 rows = 2x512 blocks)
NSB2 = H // 4        # 64 gate super-blocks
BIG = 64.0

_CACHE = {}


def _build(iters=1):
    import concourse.bacc as bacc_mod
    import concourse.tile as tile
    import concourse.mybir as mybir

    dt = mybir.dt
    nc = bacc_mod.Bacc()

    x_d = nc.dram_tensor("x", [C, H, W], dt.bfloat16, kind="ExternalInput")
    gm2_d = nc.dram_tensor("gm2", [2, NSB * 512], dt.float8e4, kind="ExternalInput")
    gt2_d = nc.dram_tensor("gt2", [2, NSB2 * 512], dt.float8e4, kind="ExternalInput")
    wd1_d = nc.dram_tensor("wd1", [128, 9, 128], dt.bfloat16, kind="ExternalInput")
    wd2_d = nc.dram_tensor("wd2", [128, 9, 128], dt.bfloat16, kind="ExternalInput")
    sb1_d = nc.dram_tensor("sb1", [128, 2], dt.float32, kind="ExternalInput")
    sb2_d = nc.dram_tensor("sb2", [128, 2], dt.float32, kind="ExternalInput")
    selb_d = nc.dram_tensor("selb", [2, 128], dt.float8e4, kind="ExternalInput")
    selp_d = nc.dram_tensor("selp", [2, 128], dt.float8e4, kind="ExternalInput")
    o_d = nc.dram_tensor("o", [C, H, W], dt.bfloat16, kind="ExternalOutput")

    RELU = mybir.ActivationFunctionType.Relu
    IDENT = mybir.ActivationFunctionType.Identity

    with tile.TileContext(nc) as tc:
        with (
            tc.tile_pool(name="const", bufs=1) as cpool,
            tc.tile_pool(name="xs", bufs=3) as xpool,
            tc.tile_pool(name="hs", bufs=3) as hpool,
            tc.tile_pool(name="msk", bufs=2) as mpool,
            tc.tile_pool(name="ov", bufs=2) as ovpool,
            tc.tile_pool(name="work", bufs=2) as wpool,
            tc.tile_pool(name="ps1", bufs=2, space="PSUM") as ps1,
            tc.tile_pool(name="ps2", bufs=3, space="PSUM") as ps2,
            tc.tile_pool(name="pm", bufs=3, space="PSUM") as pmp,
        ):
            wd1 = cpool.tile([128, 9, 128], dt.bfloat16)
            wd2 = cpool.tile([128, 9, 128], dt.bfloat16)
            sb1 = cpool.tile([128, 2], dt.float32)
            sb2 = cpool.tile([128, 2], dt.float32)
            selb = cpool.tile([2, 128], dt.float8e4)
            selp = cpool.tile([2, 128], dt.float8e4)
            for t, d in ((wd1, wd1_d), (wd2, wd2_d), (sb1, sb1_d), (sb2, sb2_d),
                         (selb, selb_d), (selp, selp_d)):
                nc.sync.dma_start(t[:], d[:])
            warm = cpool.tile([2, 64], dt.bfloat16)
            nc.vector.memset(warm[:], 0)
            wps = ps2.tile([128, 512], dt.float32, tag="ps2")
            for i in range(80):
                off = (i % 8) * 64
                nc.tensor.matmul(wps[0:64, off:off + 64], warm[:, 0:64], warm[:, :],
                                 start=True, stop=True, tile_position=(0, 0), skip_group_check=True)

            def emit_load(s):
                r0 = s * R
                first = r0 - 2
                T1 = xpool.tile([128, XR, WP], dt.bfloat16, tag="T1")
                v0 = max(0, -first)
                v1 = min(XR, H - first)
                v1u = min(XR, H - first - 2)
                nc.vector.memset(T1[:, :, 0:1], 0)
                nc.vector.memset(T1[:, :, 257:258], 0)
                if v0 > 0:
                    nc.vector.memset(T1[0:64, 0:v0, :], 0)
                if v1 < XR:
                    nc.vector.memset(T1[0:64, v1:XR, :], 0)
                if v1u < XR:
                    nc.vector.memset(T1[64:128, v1u:XR, :], 0)
                if s == 0:
                    vh = 14
                    nc.gpsimd.dma_start(T1[0:64, v0:vh, 1:257], x_d[:, first + v0:first + vh, :])
                    nc.gpsimd.dma_start(T1[64:128, 0:vh, 1:257], x_d[:, first + 2:first + 2 + vh, :])
                    nc.gpsimd.dma_start(T1[0:64, vh:v1, 1:257], x_d[:, first + vh:first + v1, :])
                    nc.gpsimd.dma_start(T1[64:128, vh:v1u, 1:257], x_d[:, first + 2 + vh:first + 2 + v1u, :])
                else:
                    nc.gpsimd.dma_start(T1[0:64, v0:v1, 1:257], x_d[:, first + v0:first + v1, :])
                    nc.gpsimd.dma_start(T1[64:128, 0:v1u, 1:257], x_d[:, first + 2:first + 2 + v1u, :])
                gmw = mpool.tile([2, NP1 * 512], dt.float8e4, tag="gmw")
                gtw = mpool.tile([2, NP2 * 512], dt.float8e4, tag="gtw")
                S0 = r0 // 4 + 1
                nc.sync.dma_start(gmw[:], gm2_d[0:2, S0 * 512:(S0 + NP1) * 512])
                nc.sync.dma_start(gtw[:], gt2_d[0:2, (r0 // 4) * 512:(r0 // 4 + NP2) * 512])
                return T1, gmw, gtw

            def new_H1():
                H1 = hpool.tile([128, HR, WP], dt.bfloat16, tag="H1")
                nc.vector.memset(H1[:, :, 0:1], 0)
                nc.vector.memset(H1[:, :, 257:258], 0)
                return H1

            def emit_conv1(s, T1, gmw, H1s):
                # computes global pairs 8s+pq for pq in [0..8] (s==0) or [1..8];
                # the boundary pair (pq==8) also writes rows 0:4 of H1(s+1).
                H1 = H1s[s]
                for pq in (range(NP1) if s == 0 else range(1, NP1)):
                    acc = ps1.tile([128, 512], dt.float32, tag="ps1")
                    for k in range(9):
                        dy, dx = k // 3, k % 3
                        tt = 4 * pq + dy
                        nc.tensor.matmul(acc[:, :], wd1[:, k, :], T1[:, tt:tt + 2, dx:dx + 256],
                                         start=(k == 0), stop=False, tile_position=(0, 0),
                                         skip_group_check=True)
                    nc.tensor.matmul(acc[:, :], selb[:, :], gmw[0:2, pq * 512:pq * 512 + 512],
                                     start=False, stop=True, tile_position=(0, 0), skip_group_check=True)
                    accv = acc[:].rearrange("p (r w) -> p r w", r=2)
                    nc.scalar.activation(H1[0:64, 4 * pq:4 * pq + 2, 1:257], accv[0:64],
                                         RELU, bias=sb1[0:64, 1:2], scale=sb1[0:64, 0:1])
                    nc.scalar.activation(H1[64:128, 4 * pq:4 * pq + 2, 1:257], accv[64:128],
                                         RELU, bias=sb1[64:128, 1:2], scale=sb1[64:128, 0:1])
                    if pq == NP1 - 1 and s + 1 < NS:
                        H1n = new_H1()
                        H1s[s + 1] = H1n
                        nc.scalar.activation(H1n[0:64, 0:2, 1:257], accv[0:64],
                                             RELU, bias=sb1[0:64, 1:2], scale=sb1[0:64, 0:1])
                        nc.scalar.activation(H1n[64:128, 0:2, 1:257], accv[64:128],
                                             RELU, bias=sb1[64:128, 1:2], scale=sb1[64:128, 0:1])
                # patch the two cross-partition quarters:
                h1l4 = H1[0:64].rearrange("c (p a) w -> c p (a w)", a=4)
                h1u4 = H1[64:128].rearrange("c (p a) w -> c p (a w)", a=4)
                nc.sync.dma_start(h1l4[:, :, 2 * WP:4 * WP], h1u4[:, :, 0:2 * WP])
                nc.sync.dma_start(h1u4[:, 0:NP1 - 1, 2 * WP:4 * WP], h1l4[:, 1:NP1, 0:2 * WP])

            def emit_conv2(s, T1, H1, gtw):
                r0 = s * R
                OV = ovpool.tile([128, NP2, 512], dt.bfloat16, tag="OV")
                for q in range(NP2):
                    pm = pmp.tile([128, 512], dt.float32, tag="pm")
                    nc.tensor.matmul(pm[:, :], selp[:, :], gtw[0:2, q * 512:q * 512 + 512],
                                     start=True, stop=True, tile_position=(0, 0), skip_group_check=True)
                    acc2 = ps2.tile([128, 512], dt.float32, tag="ps2")
                    for k in range(9):
                        dy, dx = k // 3, k % 3
                        mm = 4 * q + dy
                        nc.tensor.matmul(acc2[:, :], wd2[:, k, :], H1[:, mm:mm + 2, dx:dx + 256],
                                         start=(k == 0), stop=(k == 8), tile_position=(0, 0),
                                         skip_group_check=True)
                    u2 = wpool.tile([128, 512], dt.bfloat16, tag="u2")
                    nc.vector.tensor_scalar(u2[:], acc2[:], sb2[:, 0:1], sb2[:, 1:2],
                                            mybir.AluOpType.mult, mybir.AluOpType.add)
                    t = wpool.tile([128, 512], dt.bfloat16, tag="t")
                    nc.vector.tensor_tensor(t[:], u2[:], pm[:], mybir.AluOpType.mult)
                    v = wpool.tile([128, 512], dt.bfloat16, tag="v")
                    lz = 4 * q + 2
                    nc.vector.tensor_tensor(v[:].rearrange("p (r w) -> p r w", r=2),
                                            t[:].rearrange("p (r w) -> p r w", r=2),
                                            T1[:, lz:lz + 2, 1:257], mybir.AluOpType.add)
                    nc.scalar.activation(OV[:, q, :], v[:], RELU)
                o4 = o_d[:, r0:r0 + R, :].rearrange("c (q a) w -> c q (a w)", a=4)
                nc.sync.dma_start(o4[:, 0:4, 0:512], OV[0:64, 0:4, :])
                nc.sync.dma_start(o4[:, 0:4, 512:1024], OV[64:128, 0:4, :])
                nc.sync.dma_start(o4[:, 4:8, 0:512], OV[0:64, 4:8, :])
                nc.sync.dma_start(o4[:, 4:8, 512:1024], OV[64:128, 4:8, :])

            for it in range(iters):
                T1s = {}
                H1s = {}
                T1s[0] = emit_load(0)
                H1s[0] = new_H1()
                emit_conv1(0, T1s[0][0], T1s[0][1], H1s)
                for s in range(NS):
                    if s + 1 < NS:
                        T1s[s + 1] = emit_load(s + 1)
                        emit_conv1(s + 1, T1s[s + 1][0], T1s[s + 1][1], H1s)
                    emit_conv2(s, T1s[s][0], H1s[s], T1s[s][2])
                    T1s.pop(s)
                    H1s.pop(s)
    nc.finalize()
    return nc


def _host_prep(x, gate, w1, scale1, bias1, w2, scale2, bias2):
    # wd[k]: block-diagonal [128,128], diag blocks = wt[:, :, dy, dx] (tap k = 3*dy+dx)
    def pack(w):
        wt = np.transpose(w, (1, 0, 2, 3))  # [ci, co, dy, dx]
        wd = np.zeros((128, 9, 128), np.float32)
        for k in range(9):
            dy, dx = k // 3, k % 3
            wd[0:64, k, 0:64] = wt[:, :, dy, dx]
            wd[64:128, k, 64:128] = wt[:, :, dy, dx]
        return wd.astype(BF16)

    wd1 = pack(w1)
    wd2 = pack(w2)
    # conv1 bias folded with the -BIG gate clamp: relu(s*(acc + BIG*g) + b - s*BIG)
    sb1 = np.stack([np.tile(scale1, 2), np.tile(bias1 - scale1 * BIG, 2)], axis=1).astype(np.float32)
    sb2 = np.stack([np.tile(scale2, 2), np.tile(bias2, 2)], axis=1).astype(np.float32)

    selb = np.zeros((2, 128), np.float32)
    selb[0, 0:64] = BIG
    selb[1, 64:128] = BIG
    selp = np.zeros((2, 128), np.float32)
    selp[0, 0:64] = 1.0
    selp[1, 64:128] = 1.0
    selb = selb.astype(FP8)
    selp = selp.astype(FP8)

    g = gate[:, 0]                                   # [B, H, W]
    gp = np.pad(g, ((0, 0), (1, 1), (1, 1)))
    gm = np.zeros_like(g)
    for dy in range(3):
        for dx in range(3):
            np.maximum(gm, gp[:, dy:dy + H, dx:dx + W], out=gm)

    def blocks2(padded):                             # [rows(4k), W] -> [2, k*512]
        nsb = padded.shape[0] // 4
        arr = padded.reshape(nsb, 2, 512)
        return np.ascontiguousarray(arr.transpose(1, 0, 2)).reshape(2, -1).astype(FP8)

    gm2_l, gt2_l, xbf = [], [], []
    for bi in range(B):
        gmp = np.zeros((GROWS, W), np.float32)
        gmp[GPAD:GPAD + H] = gm[bi]
        gm2_l.append(blocks2(gmp))
        gt2_l.append(blocks2(g[bi]))
        xbf.append(np.ascontiguousarray(x[bi]).astype(BF16))
    return dict(wd1=wd1, wd2=wd2, sb1=sb1, sb2=sb2,
                selb=selb, selp=selp, gm2=gm2_l, gt2=gt2_l, x=xbf)


def _in_map(prep, bi):
    return {
        "x": prep["x"][bi],
        "gm2": prep["gm2"][bi], "gt2": prep["gt2"][bi],
        "wd1": prep["wd1"], "wd2": prep["wd2"],
        "sb1": prep["sb1"], "sb2": prep["sb2"], "selb": prep["selb"], "selp": prep["selp"],
    }


def kernel(x, gate, w1, scale1, bias1, w2, scale2, bias2):
    from concourse.bass_utils import run_bass_kernel_spmd

    x = np.asarray(x, np.float32)
    gate = np.asarray(gate, np.float32)
    prep = _host_prep(x, gate, np.asarray(w1, np.float32), np.asarray(scale1, np.float32),
                      np.asarray(bias1, np.float32), np.asarray(w2, np.float32),
                      np.asarray(scale2, np.float32), np.asarray(bias2, np.float32))

    if 'nc' not in _CACHE:
        _CACHE['nc'] = _build()
    nc = _CACHE['nc']

    in_maps = [_in_map(prep, bi) for bi in range(B)]
    res = run_bass_kernel_spmd(nc, in_maps, core_ids=list(range(B)))
    _CACHE['last_results'] = res
    out = np.stack([res.results[bi]["o"].astype(np.float32) for bi in range(B)], axis=0)
    return out


# revision 5
# speedup vs baseline: 1.8400x; 1.0054x over previous
"""Trainium2 Bass kernel v4 for masked BasicBlock (conv3x3+BN+ReLU, gated, x2, residual).

Data-parallel over batch: 8 images -> 8 NeuronCores. Per core, NCHW [64,256,256]
in 8 row-strips of 32 output rows.

Core idea (v4): every conv tap (dy,dx) is ONE [128,512] matmul with a
block-diagonal lhsT — lower 64 partitions of the rhs hold rows for output
group A, upper 64 hold rows (shifted +2) for group B, and the 64x64 tap weight
matrix sits on both diagonal blocks. 9 taps + 1 gating-selector matmul per
4-row pair, for both convs. The selector (K=2, fp8) accumulates BIG*gmax into
conv1's PSUM (ReLU with bias-BIG*scale clamps inactive pixels to 0) and
broadcasts the gate for conv2's multiply.

conv1's ACT writes h straight into the H1 conv layout (two 64-partition
activations); two strip-level SBUF DMAs patch the cross-partition quarters.
Strips are software-pipelined; x in/out are bf16 on the wire.
"""
import sys

sys.path.insert(0, '/opt/trn_rl_repo')

import numpy as np
import ml_dtypes

BF16 = ml_dtypes.bfloat16
FP8 = ml_dtypes.float8_e4m3fn

B, C, H, W = 8, 64, 256, 256
WP = W + 2           # padded row width
R = 32               # output rows per strip
NS = H // R          # strips
NP1 = (R + 4) // 4   # conv1 pairs per strip (h rows r0-1 .. r0+34)
NP2 = R // 4         # conv2 pairs per strip
XR = R + 6           # x rows per strip: [r0-2, r0+36)
HR = R + 4           # h rows per strip: [r0-1, r0+35)
GPAD = 5             # gmax pad rows on top (1 mod 4 so pair groups are 4-row aligned)
GROWS = GPAD + H + 3 # 264 gmax padded rows
NSB = GROWS // 4     # 66 gmax super-blocks (4 rows = 2x512 blocks)
NSB2 = H // 4        # 64 gate super-blocks
BIG = 64.0

_CACHE = {}


def _build(iters=1):
    import concourse.bacc as bacc_mod
    import concourse.tile as tile
    import concourse.mybir as mybir

    dt = mybir.dt
    nc = bacc_mod.Bacc()

    x_d = nc.dram_tensor("x", [C, H, W], dt.bfloat16, kind="ExternalInput")
    gm2_d = nc.dram_tensor("gm2", [2, NSB * 512], dt.float8e4, kind="ExternalInput")
    gt2_d = nc.dram_tensor("gt2", [2, NSB2 * 512], dt.float8e4, kind="ExternalInput")
    wd1_d = nc.dram_tensor("wd1", [128, 9, 128], dt.bfloat16, kind="ExternalInput")
    wd2_d = nc.dram_tensor("wd2", [128, 9, 128], dt.bfloat16, kind="ExternalInput")
    sb1_d = nc.dram_tensor("sb1", [128, 2], dt.float32, kind="ExternalInput")
    sb2_d = nc.dram_tensor("sb2", [128, 2], dt.float32, kind="ExternalInput")
    selb_d = nc.dram_tensor("selb", [2, 128], dt.float8e4, kind="ExternalInput")
    selp_d = nc.dram_tensor("selp", [2, 128], dt.float8e4, kind="ExternalInput")
    o_d = nc.dram_tensor("o", [C, H, W], dt.bfloat16, kind="ExternalOutput")

    RELU = mybir.ActivationFunctionType.Relu
    IDENT = mybir.ActivationFunctionType.Identity

    with tile.TileContext(nc) as tc:
        with (
            tc.tile_pool(name="const", bufs=1) as cpool,
            tc.tile_pool(name="xs", bufs=4) as xpool,
            tc.tile_pool(name="hs", bufs=3) as hpool,
            tc.tile_pool(name="msk", bufs=3) as mpool,
            tc.tile_pool(name="ov", bufs=2) as ovpool,
            tc.tile_pool(name="work", bufs=2) as wpool,
            tc.tile_pool(name="ps1", bufs=2, space="PSUM") as ps1,
            tc.tile_pool(name="ps2", bufs=3, space="PSUM") as ps2,
            tc.tile_pool(name="pm", bufs=3, space="PSUM") as pmp,
        ):
            wd1 = cpool.tile([128, 9, 128], dt.bfloat16)
            wd2 = cpool.tile([128, 9, 128], dt.bfloat16)
            sb1 = cpool.tile([128, 2], dt.float32)
            sb2 = cpool.tile([128, 2], dt.float32)
            selb = cpool.tile([2, 128], dt.float8e4)
            selp = cpool.tile([2, 128], dt.float8e4)
            for t, d in ((wd1, wd1_d), (wd2, wd2_d), (sb1, sb1_d), (sb2, sb2_d),
                         (selb, selb_d), (selp, selp_d)):
                nc.sync.dma_start(t[:], d[:])
            warm = cpool.tile([2, 64], dt.bfloat16)
            nc.vector.memset(warm[:], 0)
            wps = ps2.tile([128, 512], dt.float32, tag="ps2")
            for i in range(80):
                off = (i % 8) * 64
                nc.tensor.matmul(wps[0:64, off:off + 64], warm[:, 0:64], warm[:, :],
                                 start=True, stop=True, tile_position=(0, 0), skip_group_check=True)

            def emit_load(s):
                r0 = s * R
                first = r0 - 2
                T1 = xpool.tile([128, XR, WP], dt.bfloat16, tag="T1")
                v0 = max(0, -first)
                v1 = min(XR, H - first)
                v1u = min(XR, H - first - 2)
                nc.vector.memset(T1[:, :, 0:1], 0)
                nc.vector.memset(T1[:, :, 257:258], 0)
                if v0 > 0:
                    nc.vector.memset(T1[0:64, 0:v0, :], 0)
                if v1 < XR:
                    nc.vector.memset(T1[0:64, v1:XR, :], 0)
                if v1u < XR:
                    nc.vector.memset(T1[64:128, v1u:XR, :], 0)
                if s == 0:
                    vh = 14
                    nc.gpsimd.dma_start(T1[0:64, v0:vh, 1:257], x_d[:, first + v0:first + vh, :])
                    nc.gpsimd.dma_start(T1[64:128, 0:vh, 1:257], x_d[:, first + 2:first + 2 + vh, :])
                    nc.gpsimd.dma_start(T1[0:64, vh:v1, 1:257], x_d[:, first + vh:first + v1, :])
                    nc.gpsimd.dma_start(T1[64:128, vh:v1u, 1:257], x_d[:, first + 2 + vh:first + 2 + v1u, :])
                else:
                    nc.gpsimd.dma_start(T1[0:64, v0:v1, 1:257], x_d[:, first + v0:first + v1, :])
                    nc.gpsimd.dma_start(T1[64:128, 0:v1u, 1:257], x_d[:, first + 2:first + 2 + v1u, :])
                gmw = mpool.tile([2, NP1 * 512], dt.float8e4, tag="gmw")
                gtw = mpool.tile([2, NP2 * 512], dt.float8e4, tag="gtw")
                S0 = r0 // 4 + 1
                nc.sync.dma_start(gmw[:], gm2_d[0:2, S0 * 512:(S0 + NP1) * 512])
                nc.sync.dma_start(gtw[:], gt2_d[0:2, (r0 // 4) * 512:(r0 // 4 + NP2) * 512])
                return T1, gmw, gtw

            def new_H1():
                H1 = hpool.tile([128, HR, WP], dt.bfloat16, tag="H1")
                nc.vector.memset(H1[:, :, 0:1], 0)
                nc.vector.memset(H1[:, :, 257:258], 0)
                return H1

            def emit_conv1(s, T1, gmw, H1s):
                # computes global pairs 8s+pq for pq in [0..8] (s==0) or [1..8];
                # the boundary pair (pq==8) also writes rows 0:4 of H1(s+1).
                H1 = H1s[s]
                for pq in (range(NP1) if s == 0 else range(1, NP1)):
                    acc = ps1.tile([128, 512], dt.float32, tag="ps1")
                    for k in range(9):
                        dy, dx = k // 3, k % 3
                        tt = 4 * pq + dy
                        nc.tensor.matmul(acc[:, :], wd1[:, k, :], T1[:, tt:tt + 2, dx:dx + 256],
                                         start=(k == 0), stop=False, tile_position=(0, 0),
                                         skip_group_check=True)
                    nc.tensor.matmul(acc[:, :], selb[:, :], gmw[0:2, pq * 512:pq * 512 + 512],
                                     start=False, stop=True, tile_position=(0, 0), skip_group_check=True)
                    accv = acc[:].rearrange("p (r w) -> p r w", r=2)
                    nc.scalar.activation(H1[0:64, 4 * pq:4 * pq + 2, 1:257], accv[0:64],
                                         RELU, bias=sb1[0:64, 1:2], scale=sb1[0:64, 0:1])
                    nc.scalar.activation(H1[64:128, 4 * pq:4 * pq + 2, 1:257], accv[64:128],
                                         RELU, bias=sb1[64:128, 1:2], scale=sb1[64:128, 0:1])
                    if pq == NP1 - 1 and s + 1 < NS:
                        H1n = new_H1()
                        H1s[s + 1] = H1n
                        nc.scalar.activation(H1n[0:64, 0:2, 1:257], accv[0:64],
                                             RELU, bias=sb1[0:64, 1:2], scale=sb1[0:64, 0:1])
                        nc.scalar.activation(H1n[64:128, 0:2, 1:257], accv[64:128],
                                             RELU, bias=sb1[64:128, 1:2], scale=sb1[64:128, 0:1])
                # patch the two cross-partition quarters:
                h1l4 = H1[0:64].rearrange("c (p a) w -> c p (a w)", a=4)
                h1u4 = H1[64:128].rearrange("c (p a) w -> c p (a w)", a=4)
                nc.sync.dma_start(h1l4[:, :, 2 * WP:4 * WP], h1u4[:, :, 0:2 * WP])
                nc.sync.dma_start(h1u4[:, 0:NP1 - 1, 2 * WP:4 * WP], h1l4[:, 1:NP1, 0:2 * WP])

            def emit_conv2(s, T1, H1, gtw):
                r0 = s * R
                OV = ovpool.tile([128, NP2, 512], dt.bfloat16, tag="OV")
                for q in range(NP2):
                    pm = pmp.tile([128, 512], dt.float32, tag="pm")
                    nc.tensor.matmul(pm[:, :], selp[:, :], gtw[0:2, q * 512:q * 512 + 512],
                                     start=True, stop=True, tile_position=(0, 0), skip_group_check=True)
                    acc2 = ps2.tile([128, 512], dt.float32, tag="ps2")
                    for k in range(9):
                        dy, dx = k // 3, k % 3
                        mm = 4 * q + dy
                        nc.tensor.matmul(acc2[:, :], wd2[:, k, :], H1[:, mm:mm + 2, dx:dx + 256],
                                         start=(k == 0), stop=(k == 8), tile_position=(0, 0),
                                         skip_group_check=True)
                    u2 = wpool.tile([128, 512], dt.bfloat16, tag="u2")
                    nc.vector.tensor_scalar(u2[:], acc2[:], sb2[:, 0:1], sb2[:, 1:2],
                                            mybir.AluOpType.mult, mybir.AluOpType.add)
                    t = wpool.tile([128, 512], dt.bfloat16, tag="t")
                    nc.vector.tensor_tensor(t[:], u2[:], pm[:], mybir.AluOpType.mult)
                    v = wpool.tile([128, 512], dt.bfloat16, tag="v")
                    lz = 4 * q + 2
                    nc.vector.tensor_tensor(v[:].rearrange("p (r w) -> p r w", r=2),
                                            t[:].rearrange("p (r w) -> p r w", r=2),
                                            T1[:, lz:lz + 2, 1:257], mybir.AluOpType.add)
                    nc.scalar.activation(OV[:, q, :], v[:], RELU)
                o4 = o_d[:, r0:r0 + R, :].rearrange("c (q a) w -> c q (a w)", a=4)
                if s == NS - 1:
                    for qa in range(0, NP2, 2):
                        nc.sync.dma_start(o4[:, qa:qa + 2, 0:512], OV[0:64, qa:qa + 2, :])
                        nc.sync.dma_start(o4[:, qa:qa + 2, 512:1024], OV[64:128, qa:qa + 2, :])
                else:
                    nc.sync.dma_start(o4[:, 0:4, 0:512], OV[0:64, 0:4, :])
                    nc.sync.dma_start(o4[:, 0:4, 512:1024], OV[64:128, 0:4, :])
                    nc.sync.dma_start(o4[:, 4:8, 0:512], OV[0:64, 4:8, :])
                    nc.sync.dma_start(o4[:, 4:8, 512:1024], OV[64:128, 4:8, :])

            for it in range(iters):
                T1s = {}
                H1s = {}
                T1s[0] = emit_load(0)
                T1s[1] = emit_load(1)
                H1s[0] = new_H1()
                emit_conv1(0, T1s[0][0], T1s[0][1], H1s)
                for s in range(NS):
                    if s + 2 < NS:
                        T1s[s + 2] = emit_load(s + 2)
                    if s + 1 < NS:
                        emit_conv1(s + 1, T1s[s + 1][0], T1s[s + 1][1], H1s)
                    emit_conv2(s, T1s[s][0], H1s[s], T1s[s][2])
                    T1s.pop(s)
                    H1s.pop(s)
    nc.finalize()
    return nc


def _host_prep(x, gate, w1, scale1, bias1, w2, scale2, bias2):
    # wd[k]: block-diagonal [128,128], diag blocks = wt[:, :, dy, dx] (tap k = 3*dy+dx)
    def pack(w):
        wt = np.transpose(w, (1, 0, 2, 3))  # [ci, co, dy, dx]
        wd = np.zeros((128, 9, 128), np.float32)
        for k in range(9):
            dy, dx = k // 3, k % 3
            wd[0:64, k, 0:64] = wt[:, :, dy, dx]
            wd[64:128, k, 64:128] = wt[:, :, dy, dx]
        return wd.astype(BF16)

    wd1 = pack(w1)
    wd2 = pack(w2)
    # conv1 bias folded with the -BIG gate clamp: relu(s*(acc + BIG*g) + b - s*BIG)
    sb1 = np.stack([np.tile(scale1, 2), np.tile(bias1 - scale1 * BIG, 2)], axis=1).astype(np.float32)
    sb2 = np.stack([np.tile(scale2, 2), np.tile(bias2, 2)], axis=1).astype(np.float32)

    selb = np.zeros((2, 128), np.float32)
    selb[0, 0:64] = BIG
    selb[1, 64:128] = BIG
    selp = np.zeros((2, 128), np.float32)
    selp[0, 0:64] = 1.0
    selp[1, 64:128] = 1.0
    selb = selb.astype(FP8)
    selp = selp.astype(FP8)

    g = gate[:, 0]                                   # [B, H, W]
    gp = np.pad(g, ((0, 0), (1, 1), (1, 1)))
    gm = np.zeros_like(g)
    for dy in range(3):
        for dx in range(3):
            np.maximum(gm, gp[:, dy:dy + H, dx:dx + W], out=gm)

    def blocks2(padded):                             # [rows(4k), W] -> [2, k*512]
        nsb = padded.shape[0] // 4
        arr = padded.reshape(nsb, 2, 512)
        return np.ascontiguousarray(arr.transpose(1, 0, 2)).reshape(2, -1).astype(FP8)

    gm2_l, gt2_l, xbf = [], [], []
    for bi in range(B):
        gmp = np.zeros((GROWS, W), np.float32)
        gmp[GPAD:GPAD + H] = gm[bi]
        gm2_l.append(blocks2(gmp))
        gt2_l.append(blocks2(g[bi]))
        xbf.append(np.ascontiguousarray(x[bi]).astype(BF16))
    return dict(wd1=wd1, wd2=wd2, sb1=sb1, sb2=sb2,
                selb=selb, selp=selp, gm2=gm2_l, gt2=gt2_l, x=xbf)


def _in_map(prep, bi):
    return {
        "x": prep["x"][bi],
        "gm2": prep["gm2"][bi], "gt2": prep["gt2"][bi],
        "wd1": prep["wd1"], "wd2": prep["wd2"],
        "sb1": prep["sb1"], "sb2": prep["sb2"], "selb": prep["selb"], "selp": prep["selp"],
    }


def kernel(x, gate, w1, scale1, bias1, w2, scale2, bias2):
    from concourse.bass_utils import run_bass_kernel_spmd

    x = np.asarray(x, np.float32)
    gate = np.asarray(gate, np.float32)
    prep = _host_prep(x, gate, np.asarray(w1, np.float32), np.asarray(scale1, np.float32),
                      np.asarray(bias1, np.float32), np.asarray(w2, np.float32),
                      np.asarray(scale2, np.float32), np.asarray(bias2, np.float32))

    if 'nc' not in _CACHE:
        _CACHE['nc'] = _build()
    nc = _CACHE['nc']

    in_maps = [_in_map(prep, bi) for bi in range(B)]
    res = run_bass_kernel_spmd(nc, in_maps, core_ids=list(range(B)))
    _CACHE['last_results'] = res
    out = np.stack([res.results[bi]["o"].astype(np.float32) for bi in range(B)], axis=0)
    return out
